# revision 20
# baseline (speedup 1.0000x reference)
"""Trainium2 Bass kernel for DotAtten (see reference):
    qk = q[:,None,:,:] * q[:,:,None,:]; h = tanh(qk @ Wd); sjt = h @ vd
    atten = softmax(sjt, axis=2); context = atten @ value
    returns (context, atten)

Both score factors come from `query`, so sjt is SYMMETRIC in (s, t).
Each core owns 128 rows (cores 0-3 batch 0, 4-7 batch 1; data rolled so
its rows are local 0..127) and computes only the circulant half-band
t in [s, s+256) plus the shared delta=256 column, halving tensor work.
The missing half of each row is the transpose of bands computed by
peer cores: local block u in [0,128) comes from the core's own diagonal
band (local transpose), block [256,384) from peer (c+2), block
[384,512) from peer (c+3); an AllGather over each 4-core group moves
the bf16 slabs, and per-core one-hot masks (host data, keeping the SPMD
program uniform) select + merge the peer blocks via PE transposes and
predicated copies.

Matmuls keep W stationary and stream qq = q_s * q_band (512-col streams
covering two rows at once) so LDWEIGHTS pipelines behind streams; qq
builds alternate between DVE and GpSimd to keep both off the critical
path.
"""

import os
from contextlib import ExitStack

import numpy as np

B, S, E2, E = 2, 512, 512, 256
N_CORES = 8
P = 128
ROWS = P          # rows per core
HC = E2 // P      # 4 contraction chunks
DC = E // P       # 2 output-column chunks
BW = 256          # band width (delta 0..255)
TC = S // P       # 4 t-chunks for context matmul

_STATE = {}


def _build_nc(stream_dt="bf16"):
    import concourse.bacc as bacc
    import concourse.bass as bass
    import concourse.tile as tile
    from concourse import mybir
    from concourse.masks import make_identity

    f32 = mybir.dt.float32
    f32r = mybir.dt.float32r
    bf16 = mybir.dt.bfloat16
    sdt = {"bf16": bf16, "f32r": f32r}[stream_dt]

    nc = bacc.Bacc("TRN2", target_bir_lowering=False, debug=False, num_devices=8)
    qt_h = nc.dram_tensor("qt", [E2, S], f32, kind="ExternalInput")
    v_h = nc.dram_tensor("v", [S, E2], f32, kind="ExternalInput")
    w_h = nc.dram_tensor("w", [E2, E], f32, kind="ExternalInput")
    vd2_h = nc.dram_tensor("vd2", [2 * E], f32, kind="ExternalInput")
    # masks: mlt[j,u] = u<j ; mge[j,v] = v>=j (same on all cores)
    mlt_h = nc.dram_tensor("mlt", [P, P], mybir.dt.uint8, kind="ExternalInput")
    mge_h = nc.dram_tensor("mge", [P, P], mybir.dt.uint8, kind="ExternalInput")
    # per-core chunk selectors: msk2 slot i = (1-I) if i==(lc+2)%4 else 0;
    # msk3 slot i = ones if i==(lc+3)%4 else 0
    msk2_h = nc.dram_tensor("msk2", [P, 4 * P], f32, kind="ExternalInput")
    msk3_h = nc.dram_tensor("msk3", [P, 4 * P], f32, kind="ExternalInput")
    ctx_h = nc.dram_tensor("ctx_out", [ROWS, E2], f32, kind="ExternalOutput")
    att_h = nc.dram_tensor("att_out", [ROWS, S], f32, kind="ExternalOutput")
    dbg = bool(int(os.environ.get("BASS_DEBUG_DUMP", "0")))
    if dbg:
        dslab_h = nc.dram_tensor("dbg_slab", [ROWS, S], f32,
                                 kind="ExternalOutput")
        dshare_h = nc.dram_tensor("dbg_share", [4 * ROWS, S], f32,
                                  kind="ExternalOutput")

    R1 = 96   # rows in first AllGather chunk
    R2 = ROWS - R1
    slabA_h = nc.dram_tensor("slabA", [R1, S], f32)
    slabB_h = nc.dram_tensor("slabB", [R2, S], f32)
    shareA_h = nc.dram_tensor("shareA", [4 * R1, S], f32)
    shareB_h = nc.dram_tensor("shareB", [4 * R2, S], f32)

    qt, v, w = qt_h.ap(), v_h.ap(), w_h.ap()
    ctx_out, att_out = ctx_h.ap(), att_h.ap()
    GROUPS = [[0, 1, 2, 3], [4, 5, 6, 7]]

    with tile.TileContext(nc) as tc, ExitStack() as ctx:
        consts = ctx.enter_context(tc.tile_pool(name="consts", bufs=1))
        qq_pool = ctx.enter_context(tc.tile_pool(name="qq", bufs=4))
        zt_pool = ctx.enter_context(tc.tile_pool(name="zt", bufs=3))
        st_pool = ctx.enter_context(tc.tile_pool(name="st", bufs=2))
        ps_pool = ctx.enter_context(tc.tile_pool(name="ps", bufs=1, space="PSUM"))
        tail_pool = ctx.enter_context(tc.tile_pool(name="tail", bufs=1))

        # ---- constants ----
        qt_sb = consts.tile([P, HC, S], f32)
        w_f32 = consts.tile([P, HC, E], f32)
        w_sb = consts.tile([P, HC, E], sdt)
        if stream_dt == "bf16":
            qt_src = consts.tile([P, HC, S], bf16)
        else:
            qt_src = qt_sb
        for hc in range(HC):
            nc.sync.dma_start(out=w_f32[:, hc, :], in_=w[hc*P:(hc+1)*P, :])
            nc.sync.dma_start(out=qt_sb[:, hc, :], in_=qt[hc*P:(hc+1)*P, :])
            nc.vector.tensor_copy(w_sb[:, hc, :], w_f32[:, hc, :])
            if stream_dt == "bf16":
                nc.vector.tensor_copy(qt_src[:, hc, :], qt_sb[:, hc, :])
        vd_f32 = consts.tile([P, DC, 1], f32)
        vd_src = bass.AP(tensor=vd2_h, offset=0, ap=[[1, P], [P, DC]])
        nc.sync.dma_start(out=vd_f32[:, :, 0], in_=vd_src)
        vd_sb = consts.tile([P, DC, 1], sdt)
        nc.vector.tensor_copy(vd_sb[:], vd_f32[:])
        ident = consts.tile([P, P], f32)
        make_identity(nc, ident[:])
        mlt_sb = consts.tile([P, P], mybir.dt.uint8)
        mge_sb = consts.tile([P, P], mybir.dt.uint8)
        msk2_sb = consts.tile([P, 4, P], f32)
        msk3_sb = consts.tile([P, 4, P], f32)
        zsrc = consts.tile([P, S], f32)

        qtap = qt_src[:]
        if stream_dt == "f32r":
            qtap = qtap.bitcast(f32)
        pstr = qtap.ap[0][0]

        def qq_band_ap(j, stride):
            return bass.AP(tensor=qtap.tensor, offset=qtap.offset + j,
                           ap=[[pstr, P], [S, HC], [stride, BW]])

        def emit_pair(g):
            """rows j=2g, 2g+1: band matmuls + tanh + vd reduction."""
            j0 = 2 * g
            qq = qq_pool.tile([P, HC, 2, BW], sdt, tag="qq", name=f"qq_{g}")
            for r in range(2):
                j = j0 + r
                # per-hc tensor_scalar: the [P,1] scalar operand keeps the
                # op eligible for the DVE 2x 16-bit fast path
                for hc in range(HC):
                    # scalar operand must be f32 even for bf16 data
                    nc.vector.tensor_scalar_mul(
                        qq[:, hc, r, :], qt_src[:, hc, j:j+BW],
                        qt_sb[:, hc, j:j+1])
            zts = []
            for dc in range(DC):
                pz = ps_pool.tile([P, 2, BW], f32, tag=f"pz{dc}", bufs=2,
                                  name=f"pz{dc}_{g}")
                for hc in range(HC):
                    nc.tensor.matmul(pz[:],
                                     lhsT=w_sb[:, hc, dc*P:(dc+1)*P],
                                     rhs=qq[:, hc, :, :],
                                     start=(hc == 0), stop=(hc == HC-1))
                zt = zt_pool.tile([P, 2, BW], sdt, tag=f"zt{dc}", name=f"zt{dc}_{g}")
                nc.scalar.activation(zt[:], pz[:], mybir.ActivationFunctionType.Tanh)
                zts.append(zt)
            rp = ps_pool.tile([1, 2, BW], f32, tag="rp", bufs=2, name=f"rp_{g}")
            for dc in range(DC):
                nc.tensor.matmul(rp[0:1, :, :],
                                 lhsT=vd_sb[:, dc, :],
                                 rhs=zts[dc][:, :, :],
                                 start=(dc == 0), stop=(dc == DC-1))
            return rp

        # ---- main loop: 64 pairs, stage groups of 8 rows ----
        stage = None
        for g in range(ROWS // 2):
            rp = emit_pair(g)
            if g % 4 == 0:
                stage = st_pool.tile([1, 8, BW], f32, tag="stage",
                                     name=f"stage_{g}")
            sl = stage[0:1, (2*g) % 8:(2*g) % 8 + 2, :]
            nc.scalar.copy(sl, rp[:])
            if g % 4 == 3:
                j0 = 2 * g - 6  # first row of the 8-row stage group
                if j0 < R1:
                    dst = bass.AP(tensor=slabA_h, offset=j0 * (S + 1),
                                  ap=[[S + 1, 8], [1, BW]])
                else:
                    dst = bass.AP(tensor=slabB_h, offset=j0 * (S + 1) - R1 * S,
                                  ap=[[S + 1, 8], [1, BW]])
                nc.sync.dma_start(out=dst, in_=stage[0:1, :, :])
                if j0 + 8 == R1:
                    nc.gpsimd.collective_compute(
                        "AllGather", mybir.AluOpType.bypass,
                        replica_groups=GROUPS,
                        ins=[slabA_h.ap()], outs=[shareA_h.ap()])
                if j0 + 8 == ROWS:
                    nc.gpsimd.collective_compute(
                        "AllGather", mybir.AluOpType.bypass,
                        replica_groups=GROUPS,
                        ins=[slabB_h.ap()], outs=[shareB_h.ap()])
            if g == 1:
                # tail-only loads + slab zero-fill, deferred off the
                # startup critical path (DMA queues drain in issue order)
                nc.sync.dma_start(out=mlt_sb[:], in_=mlt_h.ap())
                nc.sync.dma_start(out=mge_sb[:], in_=mge_h.ap())
                nc.sync.dma_start(out=msk2_sb[:], in_=msk2_h.ap())
                nc.sync.dma_start(out=msk3_sb[:], in_=msk3_h.ap())
                # zero-fill slabs: unwritten corners otherwise hold garbage
                # that can be NaN as bf16; NaN*0 poisons the masked selection
                nc.vector.memset(zsrc[:], 0.0)
                nc.sync.dma_start(out=slabA_h.ap()[0:R1, :], in_=zsrc[0:R1, :])
                nc.sync.dma_start(out=slabB_h.ap()[0:R2, :], in_=zsrc[0:R2, :])
                v_sb = consts.tile([P, TC, E2], f32r)
                for tcc in range(TC):
                    nc.sync.dma_start(out=v_sb[:, tcc, :],
                                      in_=v[tcc*P:(tcc+1)*P, :].bitcast(f32r))

        # ---- delta=256 column: d256col[j] = score(j, j+256) ----
        qq2 = qq_pool.tile([P, HC, P], sdt, tag="qq", name="qq2")
        in0 = bass.AP(tensor=qtap.tensor, offset=qtap.offset,
                      ap=[[pstr, P], [S, HC], [1, P]])
        in1 = bass.AP(tensor=qtap.tensor, offset=qtap.offset + BW,
                      ap=[[pstr, P], [S, HC], [1, P]])
        nc.vector.tensor_tensor(out=qq2[:], in0=in0, in1=in1,
                                op=mybir.AluOpType.mult)
        zt2s = []
        for dc in range(DC):
            pz2 = ps_pool.tile([P, P], f32, tag=f"pz{dc}", bufs=2, name=f"pz2_{dc}")
            for hc in range(HC):
                nc.tensor.matmul(pz2[:],
                                 lhsT=w_sb[:, hc, dc*P:(dc+1)*P],
                                 rhs=qq2[:, hc, :],
                                 start=(hc == 0), stop=(hc == HC-1))
            zt2 = zt_pool.tile([P, P], sdt, tag=f"zt{dc}", name=f"zt2_{dc}")
            nc.scalar.activation(zt2[:], pz2[:], mybir.ActivationFunctionType.Tanh)
            zt2s.append(zt2)
        # stream the vd column twice (fp32r matmuls need even moving counts)
        d256ps = ps_pool.tile([P, 2], f32, tag="rp", bufs=2, name="d256ps")
        vdap = vd_sb[:]
        if stream_dt == "f32r":
            vdap = vdap.bitcast(f32)
        for dc in range(DC):
            vdd = bass.AP(tensor=vdap.tensor, offset=vdap.offset + dc,
                          ap=[[vdap.ap[0][0], P], [0, 2]])
            if stream_dt == "f32r":
                vdd = vdd.bitcast(f32r)
            nc.tensor.matmul(d256ps[:], lhsT=zt2s[dc][:], rhs=vdd,
                             start=(dc == 0), stop=(dc == DC-1))
        d256_sb = tail_pool.tile([P, 1], f32)
        nc.scalar.copy(d256_sb[:], d256ps[:, 0:1])
        diagt = tail_pool.tile([P, P], f32)
        nc.vector.tensor_scalar_mul(diagt[:], ident[:], d256_sb[:])

        # ---- tail: assemble full rows, softmax, context ----
        sjt = tail_pool.tile([P, S], f32)
        nc.sync.dma_start(out=sjt[0:R1, 0:3*P], in_=slabA_h.ap()[:, 0:3*P])
        nc.sync.dma_start(out=sjt[R1:P, 0:3*P], in_=slabB_h.ap()[:, 0:3*P])

        sh1b = tail_pool.tile([P, 4, P], f32)
        sh2b = tail_pool.tile([P, 4, P], f32)
        for i in range(4):
            nc.sync.dma_start(out=sh1b[0:R1, i, :],
                              in_=shareA_h.ap()[i*R1:(i+1)*R1, P:2*P])
            nc.sync.dma_start(out=sh1b[R1:P, i, :],
                              in_=shareB_h.ap()[i*R2:(i+1)*R2, P:2*P])
            nc.sync.dma_start(out=sh2b[0:R1, i, :],
                              in_=shareA_h.ap()[i*R1:(i+1)*R1, 2*P:3*P])
            nc.sync.dma_start(out=sh2b[R1:P, i, :],
                              in_=shareB_h.ap()[i*R2:(i+1)*R2, 2*P:3*P])

        if dbg:
            for part, (src_h, nr) in enumerate([(slabA_h, R1), (slabB_h, R2)]):
                df = tail_pool.tile([P, S], f32, name=f"dbg_f_{part}")
                nc.sync.dma_start(out=df[0:nr, :], in_=src_h.ap())
                nc.sync.dma_start(
                    out=dslab_h.ap()[(0 if part == 0 else R1):(R1 if part == 0 else ROWS), :],
                    in_=df[0:nr, :])
            for i in range(4):
                for part, (src_h, nr) in enumerate([(shareA_h, R1), (shareB_h, R2)]):
                    df2 = tail_pool.tile([P, S], f32, name=f"dbg2_f_{i}_{part}")
                    nc.sync.dma_start(out=df2[0:nr, :],
                                      in_=src_h.ap()[i*nr:(i+1)*nr, :])
                    base = i * ROWS + (0 if part == 0 else R1)
                    nc.sync.dma_start(out=dshare_h.ap()[base:base + nr, :],
                                      in_=df2[0:nr, :])

        # selected peer blocks (one-hot masked sum over the 4 chunks);
        # cast to f32 first: DVE tensor_tensor needs matching input dtypes
        sh1f, sh2f = sh1b, sh2b
        sel1 = tail_pool.tile([P, 4, P], f32)
        nc.vector.tensor_tensor(out=sel1[:], in0=sh1f[:], in1=msk3_sb[:],
                                op=mybir.AluOpType.mult)
        p1sel = tail_pool.tile([P, P], f32)
        nc.vector.tensor_tensor(out=p1sel[:], in0=sel1[:, 0, :], in1=sel1[:, 1, :],
                                op=mybir.AluOpType.add)
        nc.vector.tensor_tensor(out=p1sel[:], in0=p1sel[:], in1=sel1[:, 2, :],
                                op=mybir.AluOpType.add)
        nc.vector.tensor_tensor(out=p1sel[:], in0=p1sel[:], in1=sel1[:, 3, :],
                                op=mybir.AluOpType.add)
        sel2 = tail_pool.tile([P, 4, P], f32)
        nc.vector.tensor_tensor(out=sel2[:], in0=sh2f[:], in1=msk2_sb[:],
                                op=mybir.AluOpType.mult)
        p2sel = tail_pool.tile([P, P], f32)
        nc.vector.tensor_tensor(out=p2sel[:], in0=sel2[:, 0, :], in1=sel2[:, 1, :],
                                op=mybir.AluOpType.add)
        nc.vector.tensor_tensor(out=p2sel[:], in0=p2sel[:], in1=sel2[:, 2, :],
                                op=mybir.AluOpType.add)
        nc.vector.tensor_tensor(out=p2sel[:], in0=p2sel[:], in1=sel2[:, 3, :],
                                op=mybir.AluOpType.add)

        # transposes: own diag block, peer block2 (+d256 diag), peer block1
        t0ps = ps_pool.tile([P, P], f32, tag="pz0", bufs=2, name="t0ps")
        nc.tensor.transpose(t0ps[:], sjt[:, 0:P], ident[:])
        tmp0 = tail_pool.tile([P, P], f32)
        nc.scalar.copy(tmp0[:], t0ps[:])
        p2ps = ps_pool.tile([P, P], f32, tag="pz1", bufs=2, name="p2ps")
        nc.tensor.matmul(p2ps[:], lhsT=p2sel[:], rhs=ident[:],
                         is_transpose=True, start=True, stop=False)
        nc.tensor.matmul(p2ps[:], lhsT=diagt[:], rhs=ident[:],
                         is_transpose=True, start=False, stop=True)
        tmp2 = tail_pool.tile([P, P], f32)
        nc.scalar.copy(tmp2[:], p2ps[:])
        p1ps = ps_pool.tile([P, P], f32, tag="pz0", bufs=2, name="p1ps")
        nc.tensor.transpose(p1ps[:], p1sel[:], ident[:])
        nc.scalar.copy(sjt[:, 3*P:4*P], p1ps[:])

        nc.vector.copy_predicated(sjt[:, 0:P], mlt_sb[:], tmp0[:])
        nc.vector.copy_predicated(sjt[:, 2*P:3*P], mge_sb[:], tmp2[:])

        # softmax (context uses unnormalized exp; scale rows by 1/denom after)
        negmax = tail_pool.tile([P, 1], f32)
        nc.vector.reduce_max(negmax[:], sjt[:], axis=mybir.AxisListType.X,
                             negate=True)
        att = tail_pool.tile([P, S], f32)
        denom = tail_pool.tile([P, 1], f32)
        nc.scalar.activation(att[:], sjt[:], mybir.ActivationFunctionType.Exp,
                             bias=negmax[:], scale=1.0, accum_out=denom[:])
        rdenom = tail_pool.tile([P, 1], f32)
        nc.vector.reciprocal(rdenom[:], denom[:])
        atten = tail_pool.tile([P, S], f32)
        nc.vector.tensor_scalar_mul(atten[:], att[:], rdenom[:])
        nc.sync.dma_start(out=att_out[:, :], in_=atten[:])

        attT = tail_pool.tile([P, TC, P], f32r)
        for tcc in range(TC):
            pt2 = ps_pool.tile([P, P], f32, tag="pz0", bufs=2, name=f"pt2_{tcc}")
            nc.tensor.transpose(pt2[:], att[:, tcc*P:(tcc+1)*P], ident[:])
            nc.vector.tensor_copy(attT[:, tcc, :], pt2[:])
        pc = ps_pool.tile([P, E2], f32, tag="pc", bufs=1, name="pc")
        for tcc in range(TC):
            nc.tensor.matmul(pc[:], lhsT=attT[:, tcc, :], rhs=v_sb[:, tcc, :],
                             start=(tcc == 0), stop=(tcc == TC-1))
        ctx_sb = tail_pool.tile([P, E2], f32)
        nc.scalar.activation(ctx_sb[:], pc[:],
                             mybir.ActivationFunctionType.Identity,
                             scale=rdenom[:])
        nc.sync.dma_start(out=ctx_out[:, :], in_=ctx_sb[:])

    nc.compile()
    return nc


def _get_nc():
    if "nc" not in _STATE:
        _STATE["nc"] = _build_nc(os.environ.get("BASS_STREAM_DT", "bf16"))
    return _STATE["nc"]


def kernel(query, value, Wd, vd):
    from concourse.bass_utils import run_bass_kernel_spmd

    query = np.asarray(query, dtype=np.float32)
    value = np.asarray(value, dtype=np.float32)
    Wd = np.asarray(Wd, dtype=np.float32)
    vd = np.asarray(vd, dtype=np.float32)

    vd2 = np.concatenate([vd, vd])
    jj, uu = np.mgrid[0:P, 0:P]
    mlt = (uu < jj).astype(np.uint8)
    mge = (uu >= jj).astype(np.uint8)
    in_maps = []
    for c in range(N_CORES):
        b, s0 = divmod(c * ROWS, S)
        lc = c % 4
        qt = np.ascontiguousarray(np.roll(query[b].T, -s0, axis=1))
        vr = np.ascontiguousarray(np.roll(value[b], -s0, axis=0))
        msk2 = np.zeros((P, 4, P), np.float32)
        msk2[:, (lc + 2) % 4, :] = 1.0 - np.eye(P, dtype=np.float32)
        msk3 = np.zeros((P, 4, P), np.float32)
        msk3[:, (lc + 3) % 4, :] = 1.0
        in_maps.append({"qt": qt, "v": vr, "w": Wd, "vd2": vd2,
                        "mlt": mlt, "mge": mge,
                        "msk2": msk2.reshape(P, 4 * P),
                        "msk3": msk3.reshape(P, 4 * P)})

    nc = _get_nc()
    trace = bool(int(os.environ.get("BASS_KERNEL_TRACE", "0")))
    res = run_bass_kernel_spmd(nc, in_maps, list(range(N_CORES)), trace=trace)
    _STATE["last_result"] = res

    context = np.empty((B, S, E2), np.float32)
    atten = np.empty((B, S, S), np.float32)
    for c in range(N_CORES):
        b, s0 = divmod(c * ROWS, S)
        context[b, s0:s0 + ROWS] = res.results[c]["ctx_out"]
        atten[b, s0:s0 + ROWS] = np.roll(res.results[c]["att_out"], s0, axis=1)
    return context, atten
